# revision 43
# baseline (speedup 1.0000x reference)
"""Trainium2 Bass kernel: fused concat-linear attention map + softmax.

reference:  scores[b,h,n] = key[b,n,:]@Wk[h,:] + query[b,0,:]@Wq[h,:] + bias[h]
            attn = softmax over n              (B=16, N=20000, D=256, H=8)

v7 structure (per core = 2 batches, each 20000 rows):
  * tiny operands (bf16 identity, zero-padded WkT stationaries, q@Wq+b
    biases, fold32 matrix) are precomputed on the host and DMA'd in.
  * p-MAJOR cast-DMA key loads (SWDGE, f32->bf16 during the HBM read):
    "(p s) d" puts 64 consecutive rows on one partition -> 16KB-chunk
    contiguous HBM reads per partition (vs 1KB in n-interleaved order),
    measurably faster wire (~6-8us/core).  The resulting within-set
    n-permutation (n = 64p + 4g + j) is undone on the HOST during the
    gather/unshard step (pure reshape/transpose, no math).
  * every set-load is split into 2 half-DMAs (first set: 4 quarters)
    so consumers wake on partial data; shortens ramp and tail.
  * 128x128 key-tile transposes as REGULAR bf16 matmuls vs bf16 identity
    (pipelined LDW+MM, FWL; avoids transpose-mode stalls).
  * PSUM->SBUF copies (f32 -> round bf16) split DVE/ACT by ratio.
  * score matmuls use ZERO-PADDED stationaries wk32[dhalf][g] ([128,128],
    cols 8g..8g+8 = WkT half): SIXTEEN 512-col sub-chunks accumulate into
    ONE [128,512] PSUM bank at partition groups g=0..15 (8192 rows/set).
    ACT/DVE op cost depends on FREE size only, so stacking 16 groups on
    the partition axis cuts exp/reduce/scale work 4x vs G=4 -- all
    vector-engine pressure drops below the wire and the batch-boundary
    store burst shrinks to 4 tasks.  (G=4 -> 8 -> 16 measured ~139.5 ->
    132.6 -> 131.3 us fast-core.)
  * score MMs are emitted 2 groups BEHIND their transposes (the in-order
    Tensor engine otherwise stalls on each group's PSUM->SBUF copy).
  * totals: ONE fold32 [32,32] matmul replaces the foldA/foldB round
    trip; reciprocal reads the PSUM result directly; the last set's exp
    accumulates its sum in-instruction (accum_out) to shorten the chain.
  * final scale muls alternate DVE/ACT; store DMAs round-robin
    sync/gpsimd/scalar on the last batch so the tail is not
    issue-serialized on one engine.  (gpsimd/Pool must NEVER run
    tensor math here: its DSP path is ~10x slower and numerically
    broke tensor_scalar in testing.)
"""

import sys

import numpy as np

for _p in ("/opt/trn_rl_repo",):
    if _p not in sys.path:
        sys.path.append(_p)

from contextlib import ExitStack

import ml_dtypes
import concourse.bass as bass
import concourse.bacc as bacc
import concourse.tile as tile
from concourse import mybir

B, N, D, H = 16, 20000, 256, 8
NCORES = 8
BPC = B // NCORES
P = 128
G = 16               # partition groups stacked in score PSUM
HG = H * G           # 32
F32 = mybir.dt.float32
BF16 = mybir.dt.bfloat16
NP_BF16 = ml_dtypes.bfloat16

# per-batch plan: 4 full loads (S=32 subtiles = 4096 rows), one S=28 load,
# then a 32-row tail handled separately.
FULL_LOADS = 2
PART_S = 28          # subtiles in the partial load (3584 rows)
SROWS = 8192         # rows per full set (G=16 partition groups stacked)
TAIL_N0 = FULL_LOADS * SROWS + PART_S * 128  # 19968
TAIL_ROWS = N - TAIL_N0                      # 32
NSETS = FULL_LOADS + 1                       # score psum sets per batch
ACT_COPY_FRAC = 0.44  # fraction of PSUM->SBUF copies done on ACT


def build_kernel(bpc=BPC, reps=1):
    nc = bacc.Bacc("TRN2", target_bir_lowering=False, debug=False)
    k_in = nc.declare_dram_parameter("k", [bpc, N, D], F32, isOutput=False)
    id_in = nc.declare_dram_parameter("idbf", [P, P], BF16, isOutput=False)
    w32_in = nc.declare_dram_parameter("w32", [P, 2 * G * HG], BF16, isOutput=False)
    qb32_in = nc.declare_dram_parameter("qb32", [HG, bpc], F32, isOutput=False)
    fold32_in = nc.declare_dram_parameter("fold32", [HG, HG], F32, isOutput=False)
    out = nc.declare_dram_parameter("out", [bpc, H, N], F32, isOutput=True)

    with ExitStack() as ctx:
        tc = ctx.enter_context(tile.TileContext(nc))
        consts = ctx.enter_context(tc.tile_pool(name="consts", bufs=1))
        loads = ctx.enter_context(tc.tile_pool(name="loads", bufs=5))
        kts = ctx.enter_context(tc.tile_pool(name="kts", bufs=4))
        probp = ctx.enter_context(tc.tile_pool(name="prob", bufs=2))
        small = ctx.enter_context(tc.tile_pool(name="small", bufs=2))
        psum_kt = ctx.enter_context(tc.tile_pool(name="psum_kt", bufs=3, space="PSUM"))
        psum_sc = ctx.enter_context(tc.tile_pool(name="psum_sc", bufs=2, space="PSUM"))

        id_bf = consts.tile([P, P], BF16)
        nc.sync.dma_start(out=id_bf[:, :], in_=id_in[:, :])
        wk32 = consts.tile([P, 2, G, HG], BF16)
        nc.sync.dma_start(
            out=wk32[:, :, :, :],
            in_=w32_in[:, :].rearrange("p (d g x) -> p d g x", d=2, g=G),
        )
        qb32 = consts.tile([HG, bpc], F32)
        nc.sync.dma_start(out=qb32[:, :], in_=qb32_in[:, :])
        fold32 = consts.tile([HG, HG], F32)
        nc.sync.dma_start(out=fold32[:, :], in_=fold32_in[:, :])

        act_period = max(2, round(1.0 / max(ACT_COPY_FRAC, 1e-6)))
        copy_idx = [0]

        def copy_out(dst, src):
            i = copy_idx[0]
            copy_idx[0] += 1
            # first ~1.5 sets: all-DVE so ACT reaches the first exp sooner
            if i >= 12 and i % act_period == 0:
                nc.scalar.copy(out=dst, in_=src)
            else:
                nc.vector.tensor_copy(out=dst, in_=src)

        # --- main loop -----------------------------------------------------
        first_iter = [True]
        pend = []  # deferred score MMs: (scp, m, g, ng, k0, k1, prob32, sums, i)

        def emit_score(e):
            scp_, m_, g_, ng_, k0_, k1_, prob_, sums_, ib_ = e
            nc.tensor.matmul(
                scp_[:, :], wk32[:, 0, g_, :], k0_[:, :],
                start=(g_ == 0), stop=False,
            )
            nc.tensor.matmul(
                scp_[:, :], wk32[:, 1, g_, :], k1_[:, :],
                start=False, stop=(g_ == ng_ - 1),
            )
            if g_ == ng_ - 1:
                hg = ng_ * H
                if m_ == NSETS - 1:
                    # last set: accumulate the sum inside the exp so the
                    # totals chain starts immediately (critical path)
                    nc.scalar.activation(
                        out=prob_[:hg, 512 * m_:512 * (m_ + 1)],
                        in_=scp_[:hg, :],
                        func=mybir.ActivationFunctionType.Exp,
                        bias=qb32[:hg, ib_:ib_ + 1],
                        scale=1.0,
                        accum_out=sums_[:hg, m_:m_ + 1],
                    )
                else:
                    nc.scalar.activation(
                        out=prob_[:hg, 512 * m_:512 * (m_ + 1)],
                        in_=scp_[:hg, :],
                        func=mybir.ActivationFunctionType.Exp,
                        bias=qb32[:hg, ib_:ib_ + 1],
                        scale=1.0,
                    )
                    nc.vector.reduce_sum(
                        out=sums_[:hg, m_:m_ + 1],
                        in_=prob_[:hg, 512 * m_:512 * (m_ + 1)],
                        axis=mybir.AxisListType.X,
                    )

        for i in [ib for _ in range(reps) for ib in range(bpc)]:
            last_batch = i == bpc - 1
            # prob32[8g+h, 512m+c] = attn[h, perm(n)] (pre-scale)
            prob32 = probp.tile([HG, 512 * NSETS], F32, tag="prob")
            probT = probp.tile([H, TAIL_ROWS], F32, tag="probT")
            sums = small.tile([HG, NSETS + 1], F32, tag="sums")
            nc.vector.memset(sums[:, :], 0)

            for m in range(NSETS):
                S = 64 if m < FULL_LOADS else PART_S
                n0 = m * SROWS
                rows = S * P
                kb = loads.tile([P, 64, 2, P], BF16, tag="load")
                # p-major source view: partition p holds rows n0+S*p..+S-1
                src = k_in[i, n0:n0 + rows, :].rearrange("(p s) d -> p s d", p=P)
                if first_iter[0]:
                    first_iter[0] = False
                    bounds = [0, 4, 8, 16, 32, 64]  # fine-grained: short ramp
                elif S != 64:
                    bounds = [0, 10, 19, 28]    # partial set: 3 chunks
                else:
                    bounds = [0, 16, 32, 48, 64]  # quarters elsewhere
                for q in range(len(bounds) - 1):
                    nc.gpsimd.dma_start(
                        out=kb[:, bounds[q]:bounds[q + 1], :, :],
                        in_=src[:, bounds[q]:bounds[q + 1], :],
                    )
                ng = S // 4  # 512-col groups in this set (4 or 3)
                scp = psum_sc.tile([HG, 512], F32, tag="sc")
                for g in range(ng):
                    kt0 = psum_kt.tile([P, 512], F32, tag="kt0")
                    kt1 = psum_kt.tile([P, 512], F32, tag="kt1")
                    for t in range(4):
                        s = g * 4 + t
                        nc.tensor.matmul(
                            kt0[:, t * P:(t + 1) * P],
                            kb[:, s, 0, :],
                            id_bf[:, :],
                            start=True,
                            stop=True,
                        )
                        nc.tensor.matmul(
                            kt1[:, t * P:(t + 1) * P],
                            kb[:, s, 1, :],
                            id_bf[:, :],
                            start=True,
                            stop=True,
                        )
                    k0 = kts.tile([P, 512], BF16, tag="k0")
                    k1 = kts.tile([P, 512], BF16, tag="k1")
                    copy_out(k0[:, :], kt0[:, :])
                    copy_out(k1[:, :], kt1[:, :])
                    # defer the score MMs by 2 groups: the in-order Tensor
                    # engine would otherwise stall on each group's PSUM->SBUF
                    # copy (~0.3us x 80 groups); with the delay the copy
                    # completes during the next 2 groups' transposes.
                    pend.append((scp, m, g, ng, k0, k1, prob32, sums, i))
                    if len(pend) > 2:
                        emit_score(pend.pop(0))


            # ---- 32-row tail (natural n order) ----------------------------
            kbt = loads.tile([P, 16, 2, P], BF16, tag="load")
            nc.gpsimd.dma_start(
                out=kbt[:TAIL_ROWS, :1, :, :],
                in_=k_in[i, TAIL_N0:N, :].rearrange("(s p) d -> p s d", p=TAIL_ROWS),
            )
            ktt0 = psum_kt.tile([P, 512], F32, tag="kt0")
            ktt1 = psum_kt.tile([P, 512], F32, tag="kt1")
            nc.tensor.matmul(
                ktt0[:, :TAIL_ROWS], kbt[:TAIL_ROWS, 0, 0, :],
                id_bf[:TAIL_ROWS, :TAIL_ROWS], start=True, stop=True,
            )
            nc.tensor.matmul(
                ktt1[:, :TAIL_ROWS], kbt[:TAIL_ROWS, 0, 1, :],
                id_bf[:TAIL_ROWS, :TAIL_ROWS], start=True, stop=True,
            )
            kt_s = kts.tile([P, 512], BF16, tag="k0")
            copy_out(kt_s[:, :TAIL_ROWS], ktt0[:, :TAIL_ROWS])
            copy_out(kt_s[:, 64:64 + TAIL_ROWS], ktt1[:, :TAIL_ROWS])
            # flush the deferred score MMs (the tail transposes above keep
            # the Tensor engine fed while the last copies complete)
            while pend:
                emit_score(pend.pop(0))
            sct = psum_sc.tile([HG, 512], F32, tag="sc")
            nc.tensor.matmul(
                sct[:H, :TAIL_ROWS], wk32[:, 0, 0, :H], kt_s[:, :TAIL_ROWS],
                start=True, stop=False,
            )
            nc.tensor.matmul(
                sct[:H, :TAIL_ROWS], wk32[:, 1, 0, :H],
                kt_s[:, 64:64 + TAIL_ROWS], start=False, stop=True,
            )
            nc.scalar.activation(
                out=probT[:, :],
                in_=sct[:H, :TAIL_ROWS],
                func=mybir.ActivationFunctionType.Exp,
                bias=qb32[:H, i:i + 1],
                scale=1.0,
                accum_out=sums[:H, NSETS:NSETS + 1],
            )

            # ---- totals, scale, store -------------------------------------
            srow = small.tile([HG, 1], F32, tag="srow")
            nc.vector.reduce_sum(out=srow[:, :], in_=sums[:, :], axis=mybir.AxisListType.X)
            totp = psum_sc.tile([HG, 512], F32, tag="sc")
            nc.tensor.matmul(totp[:, :1], fold32[:, :], srow[:, :])
            rec32 = small.tile([HG, 1], F32, tag="rec32")
            nc.vector.reciprocal(out=rec32[:, :], in_=totp[:, :1])

            # Scale+store as one task per 2048-row segment.  For the first
            # batch the tasks are DEFERRED and dripped one-per-set through
            # the next batch's loop (dumping 11 muls + 11 stores at once
            # collides with the next batch's copies/exps and stalls it by
            # ~10us).  The last batch runs them immediately, muls round-
            # robin DVE/ACT/Pool and stores round-robin sync/gpsimd/scalar.
            if last_batch:
                mul_engs = [
                    lambda seg, np_, r=rec32: nc.vector.tensor_scalar_mul(
                        seg, seg, r[:np_, :]),
                    lambda seg, np_, r=rec32: nc.scalar.mul(seg, seg, r[:np_, :1]),
                ]
                st_engs = [nc.sync, nc.gpsimd]
            else:
                mul_engs = [
                    lambda seg, np_, r=rec32: nc.vector.tensor_scalar_mul(
                        seg, seg, r[:np_, :]),
                    lambda seg, np_, r=rec32: nc.scalar.mul(seg, seg, r[:np_, :1]),
                ]
                st_engs = [nc.sync]
            pg = PART_S // 4  # 3 groups in the partial set
            full_cols = 512 * FULL_LOADS  # 4608

            def seg_task(mi, i_=i, prob_=prob32, probT_=probT, rec_=rec32,
                         mul_engs_=mul_engs, st_engs_=st_engs):
                if mi < FULL_LOADS:
                    seg = prob_[:, 512 * mi:512 * (mi + 1)]
                    mul_engs_[mi % len(mul_engs_)](seg, HG)
                    st_engs_[mi % len(st_engs_)].dma_start(
                        out=out[i_, :, SROWS * mi:SROWS * (mi + 1)].rearrange(
                            "h (g c) -> g h c", c=512
                        ),
                        in_=seg,
                    )
                elif mi == FULL_LOADS:
                    segp = prob_[:H * pg, full_cols:full_cols + 512]
                    mul_engs_[mi % len(mul_engs_)](segp, H * pg)
                    st_engs_[mi % len(st_engs_)].dma_start(
                        out=out[i_, :, SROWS * FULL_LOADS:TAIL_N0].rearrange(
                            "h (g c) -> g h c", c=512
                        ),
                        in_=segp,
                    )
                else:
                    nc.scalar.mul(probT_[:, :], probT_[:, :], rec_[:H, :1])
                    st_engs_[mi % len(st_engs_)].dma_start(
                        out=out[i_, :, TAIL_N0:], in_=probT_[:, :]
                    )

            for mi in range(FULL_LOADS + 2):
                seg_task(mi)

    nc.compile()
    return nc


_NC_CACHE = {}


def _get_nc():
    if "nc" not in _NC_CACHE:
        _NC_CACHE["nc"] = build_kernel()
    return _NC_CACHE["nc"]


def make_in_maps(query, key, W, b):
    """Host-side precompute of the tiny operands + per-core input maps."""
    query = np.asarray(query, np.float32).reshape(B, D)
    key = np.ascontiguousarray(np.asarray(key, np.float32))
    W = np.asarray(W, np.float32)
    b = np.asarray(b, np.float32)

    Wq, Wk = W[:, :D], W[:, D:]                      # [H, D] each
    qb_all = query @ Wq.T + b[None, :]               # [B, H]
    WkT = Wk.T.astype(NP_BF16)                       # [D, H] bf16
    # wk32[p, dhalf, g, 8g+h] = WkT[dhalf*128 + p, h]
    wk32 = np.zeros((P, 2, G, HG), NP_BF16)
    for g in range(G):
        wk32[:, 0, g, g * H:(g + 1) * H] = WkT[:P]
        wk32[:, 1, g, g * H:(g + 1) * H] = WkT[P:]
    idbf = np.eye(P, dtype=NP_BF16)
    fold32 = np.tile(np.eye(H, dtype=np.float32), (G, G))       # [HG, HG]

    in_maps = []
    for c in range(NCORES):
        s = slice(BPC * c, BPC * (c + 1))
        qb32 = np.tile(np.ascontiguousarray(qb_all[s].T), (G, 1))  # [HG, bpc]
        in_maps.append(
            {
                "k": key[s],
                "idbf": idbf,
                "w32": np.ascontiguousarray(wk32.reshape(P, 2 * G * HG)),
                "qb32": np.ascontiguousarray(qb32),
                "fold32": fold32,
            }
        )
    return in_maps


def unpermute(raw):
    """Undo the p-major within-set n-permutation (device col -> true n).

    Full sets: dev col SROWS*m+512g+128j+p holds true n SROWS*m+64p+4g+j.
    Partial:   dev col 16384+512g+128j+p  holds true n 16384+28p+4g+j.
    Tail (32): already in true order.
    """
    nb, nh = raw.shape[0], raw.shape[1]
    out = np.empty_like(raw)
    blk = raw[:, :, : FULL_LOADS * SROWS].reshape(nb, nh, FULL_LOADS, G, 4, P)
    out[:, :, : FULL_LOADS * SROWS] = blk.transpose(0, 1, 2, 5, 3, 4).reshape(
        nb, nh, FULL_LOADS * SROWS
    )
    pb = raw[:, :, FULL_LOADS * SROWS:TAIL_N0].reshape(nb, nh, PART_S // 4, 4, P)
    out[:, :, FULL_LOADS * SROWS:TAIL_N0] = pb.transpose(0, 1, 4, 2, 3).reshape(
        nb, nh, PART_S * P
    )
    out[:, :, TAIL_N0:] = raw[:, :, TAIL_N0:]
    return out


def kernel(query, key, W, b):
    from concourse.bass_utils import run_bass_kernel_spmd

    nc = _get_nc()
    in_maps = make_in_maps(query, key, W, b)
    res = run_bass_kernel_spmd(nc, in_maps, list(range(NCORES))).results
    raw = np.concatenate([res[c]["out"] for c in range(NCORES)], axis=0)
    return unpermute(raw)


# revision 44
# speedup vs baseline: 1.0092x; 1.0092x over previous
"""Trainium2 Bass kernel: fused concat-linear attention map + softmax.

reference:  scores[b,h,n] = key[b,n,:]@Wk[h,:] + query[b,0,:]@Wq[h,:] + bias[h]
            attn = softmax over n              (B=16, N=20000, D=256, H=8)

v7 structure (per core = 2 batches, each 20000 rows):
  * tiny operands (bf16 identity, zero-padded WkT stationaries, q@Wq+b
    biases, fold32 matrix) are precomputed on the host and DMA'd in.
  * p-MAJOR cast-DMA key loads (SWDGE, f32->bf16 during the HBM read):
    "(p s) d" puts 64 consecutive rows on one partition -> 16KB-chunk
    contiguous HBM reads per partition (vs 1KB in n-interleaved order),
    measurably faster wire (~6-8us/core).  The resulting within-set
    n-permutation (n = 64p + 4g + j) is undone on the HOST during the
    gather/unshard step (pure reshape/transpose, no math).
  * every set-load is split into 2 half-DMAs (first set: 4 quarters)
    so consumers wake on partial data; shortens ramp and tail.
  * 128x128 key-tile transposes as REGULAR bf16 matmuls vs bf16 identity
    (pipelined LDW+MM, FWL; avoids transpose-mode stalls).
  * PSUM->SBUF copies (f32 -> round bf16) split DVE/ACT by ratio.
  * score matmuls use ZERO-PADDED stationaries wk32[dhalf][g] ([128,128],
    cols 8g..8g+8 = WkT half): SIXTEEN 512-col sub-chunks accumulate into
    ONE [128,512] PSUM bank at partition groups g=0..15 (8192 rows/set).
    ACT/DVE op cost depends on FREE size only, so stacking 16 groups on
    the partition axis cuts exp/reduce/scale work 4x vs G=4 -- all
    vector-engine pressure drops below the wire and the batch-boundary
    store burst shrinks to 4 tasks.  (G=4 -> 8 -> 16 measured ~139.5 ->
    132.6 -> 131.3 us fast-core.)
  * score MMs are emitted 2 groups BEHIND their transposes (the in-order
    Tensor engine otherwise stalls on each group's PSUM->SBUF copy).
  * totals: ONE fold32 [32,32] matmul replaces the foldA/foldB round
    trip; reciprocal reads the PSUM result directly; the last set's exp
    accumulates its sum in-instruction (accum_out) to shorten the chain.
  * final scale muls alternate DVE/ACT; store DMAs round-robin
    sync/gpsimd/scalar on the last batch so the tail is not
    issue-serialized on one engine.  (gpsimd/Pool must NEVER run
    tensor math here: its DSP path is ~10x slower and numerically
    broke tensor_scalar in testing.)
"""

import sys

import numpy as np

for _p in ("/opt/trn_rl_repo",):
    if _p not in sys.path:
        sys.path.append(_p)

from contextlib import ExitStack

import ml_dtypes
import concourse.bass as bass
import concourse.bacc as bacc
import concourse.tile as tile
from concourse import mybir

B, N, D, H = 16, 20000, 256, 8
NCORES = 8
BPC = B // NCORES
P = 128
G = 16               # partition groups stacked in score PSUM
HG = H * G           # 32
F32 = mybir.dt.float32
BF16 = mybir.dt.bfloat16
NP_BF16 = ml_dtypes.bfloat16

# per-batch plan: 4 full loads (S=32 subtiles = 4096 rows), one S=28 load,
# then a 32-row tail handled separately.
FULL_LOADS = 2
PART_S = 28          # subtiles in the partial load (3584 rows)
SROWS = 8192         # rows per full set (G=16 partition groups stacked)
TAIL_N0 = FULL_LOADS * SROWS + PART_S * 128  # 19968
TAIL_ROWS = N - TAIL_N0                      # 32
NSETS = FULL_LOADS + 1                       # score psum sets per batch
ACT_COPY_FRAC = 0.44  # fraction of PSUM->SBUF copies done on ACT


def build_kernel(bpc=BPC, reps=1):
    nc = bacc.Bacc("TRN2", target_bir_lowering=False, debug=False)
    k_in = nc.declare_dram_parameter("k", [bpc, N, D], F32, isOutput=False)
    id_in = nc.declare_dram_parameter("idbf", [P, P], BF16, isOutput=False)
    w32_in = nc.declare_dram_parameter("w32", [P, 2 * G * HG], BF16, isOutput=False)
    qb32_in = nc.declare_dram_parameter("qb32", [HG, bpc], F32, isOutput=False)
    fold32_in = nc.declare_dram_parameter("fold32", [HG, HG], F32, isOutput=False)
    out = nc.declare_dram_parameter("out", [bpc, H, N], F32, isOutput=True)

    with ExitStack() as ctx:
        tc = ctx.enter_context(tile.TileContext(nc))
        consts = ctx.enter_context(tc.tile_pool(name="consts", bufs=1))
        loads = ctx.enter_context(tc.tile_pool(name="loads", bufs=5))
        kts = ctx.enter_context(tc.tile_pool(name="kts", bufs=4))
        probp = ctx.enter_context(tc.tile_pool(name="prob", bufs=2))
        small = ctx.enter_context(tc.tile_pool(name="small", bufs=2))
        psum_kt = ctx.enter_context(tc.tile_pool(name="psum_kt", bufs=3, space="PSUM"))
        psum_sc = ctx.enter_context(tc.tile_pool(name="psum_sc", bufs=2, space="PSUM"))

        id_bf = consts.tile([P, P], BF16)
        nc.sync.dma_start(out=id_bf[:, :], in_=id_in[:, :])
        wk32 = consts.tile([P, 2, G, HG], BF16)
        nc.sync.dma_start(
            out=wk32[:, :, :, :],
            in_=w32_in[:, :].rearrange("p (d g x) -> p d g x", d=2, g=G),
        )
        qb32 = consts.tile([HG, bpc], F32)
        nc.sync.dma_start(out=qb32[:, :], in_=qb32_in[:, :])
        fold32 = consts.tile([HG, HG], F32)
        nc.sync.dma_start(out=fold32[:, :], in_=fold32_in[:, :])

        act_period = max(2, round(1.0 / max(ACT_COPY_FRAC, 1e-6)))
        copy_idx = [0]

        def copy_out(dst, src):
            i = copy_idx[0]
            copy_idx[0] += 1
            # first ~1.5 sets: all-DVE so ACT reaches the first exp sooner
            if i >= 12 and i % act_period == 0:
                nc.scalar.copy(out=dst, in_=src)
            else:
                nc.vector.tensor_copy(out=dst, in_=src)

        # --- main loop -----------------------------------------------------
        first_iter = [True]
        pend = []  # deferred score MMs: (scp, m, g, ng, k0, k1, prob32, sums, i)

        def emit_score(e):
            scp_, m_, g_, ng_, k0_, k1_, prob_, sums_, ib_ = e
            nc.tensor.matmul(
                scp_[:, :], wk32[:, 0, g_, :], k0_[:, :],
                start=(g_ == 0), stop=False,
            )
            nc.tensor.matmul(
                scp_[:, :], wk32[:, 1, g_, :], k1_[:, :],
                start=False, stop=(g_ == ng_ - 1),
            )
            if g_ == ng_ - 1:
                hg = ng_ * H
                if m_ == NSETS - 1:
                    # last set: accumulate the sum inside the exp so the
                    # totals chain starts immediately (critical path)
                    nc.scalar.activation(
                        out=prob_[:hg, 512 * m_:512 * (m_ + 1)],
                        in_=scp_[:hg, :],
                        func=mybir.ActivationFunctionType.Exp,
                        bias=qb32[:hg, ib_:ib_ + 1],
                        scale=1.0,
                        accum_out=sums_[:hg, m_:m_ + 1],
                    )
                else:
                    nc.scalar.activation(
                        out=prob_[:hg, 512 * m_:512 * (m_ + 1)],
                        in_=scp_[:hg, :],
                        func=mybir.ActivationFunctionType.Exp,
                        bias=qb32[:hg, ib_:ib_ + 1],
                        scale=1.0,
                    )
                    nc.vector.reduce_sum(
                        out=sums_[:hg, m_:m_ + 1],
                        in_=prob_[:hg, 512 * m_:512 * (m_ + 1)],
                        axis=mybir.AxisListType.X,
                    )

        for i in [ib for _ in range(reps) for ib in range(bpc)]:
            last_batch = i == bpc - 1
            # prob32[8g+h, 512m+c] = attn[h, perm(n)] (pre-scale)
            prob32 = probp.tile([HG, 512 * NSETS], F32, tag="prob")
            probT = probp.tile([H, TAIL_ROWS], F32, tag="probT")
            sums = small.tile([HG, NSETS + 1], F32, tag="sums")
            nc.vector.memset(sums[:, :], 0)

            for m in range(NSETS):
                S = 64 if m < FULL_LOADS else PART_S
                n0 = m * SROWS
                rows = S * P
                kb = loads.tile([P, 64, 2, P], BF16, tag="load")
                # p-major source view: partition p holds rows n0+S*p..+S-1
                src = k_in[i, n0:n0 + rows, :].rearrange("(p s) d -> p s d", p=P)
                if first_iter[0]:
                    first_iter[0] = False
                    bounds = [0, 4, 8, 16, 32, 64]  # fine-grained: short ramp
                elif S != 64:
                    bounds = [0, 10, 19, 28]    # partial set: 3 chunks
                else:
                    bounds = [0, 16, 32, 48, 64]  # quarters elsewhere
                for q in range(len(bounds) - 1):
                    nc.gpsimd.dma_start(
                        out=kb[:, bounds[q]:bounds[q + 1], :, :],
                        in_=src[:, bounds[q]:bounds[q + 1], :],
                    )
                ng = S // 4  # 512-col groups in this set (4 or 3)
                scp = psum_sc.tile([HG, 512], F32, tag="sc")
                for g in range(ng):
                    kt0 = psum_kt.tile([P, 512], F32, tag="kt0")
                    kt1 = psum_kt.tile([P, 512], F32, tag="kt1")
                    for t in range(4):
                        s = g * 4 + t
                        nc.tensor.matmul(
                            kt0[:, t * P:(t + 1) * P],
                            kb[:, s, 0, :],
                            id_bf[:, :],
                            start=True,
                            stop=True,
                        )
                        nc.tensor.matmul(
                            kt1[:, t * P:(t + 1) * P],
                            kb[:, s, 1, :],
                            id_bf[:, :],
                            start=True,
                            stop=True,
                        )
                    k0 = kts.tile([P, 512], BF16, tag="k0")
                    k1 = kts.tile([P, 512], BF16, tag="k1")
                    copy_out(k0[:, :], kt0[:, :])
                    copy_out(k1[:, :], kt1[:, :])
                    # defer the score MMs by 2 groups: the in-order Tensor
                    # engine would otherwise stall on each group's PSUM->SBUF
                    # copy (~0.3us x 80 groups); with the delay the copy
                    # completes during the next 2 groups' transposes.
                    pend.append((scp, m, g, ng, k0, k1, prob32, sums, i))
                    if len(pend) > 2:
                        emit_score(pend.pop(0))


            # ---- 32-row tail (natural n order) ----------------------------
            kbt = loads.tile([P, 16, 2, P], BF16, tag="load")
            nc.gpsimd.dma_start(
                out=kbt[:TAIL_ROWS, :1, :, :],
                in_=k_in[i, TAIL_N0:N, :].rearrange("(s p) d -> p s d", p=TAIL_ROWS),
            )
            ktt0 = psum_kt.tile([P, 512], F32, tag="kt0")
            ktt1 = psum_kt.tile([P, 512], F32, tag="kt1")
            nc.tensor.matmul(
                ktt0[:, :TAIL_ROWS], kbt[:TAIL_ROWS, 0, 0, :],
                id_bf[:TAIL_ROWS, :TAIL_ROWS], start=True, stop=True,
            )
            nc.tensor.matmul(
                ktt1[:, :TAIL_ROWS], kbt[:TAIL_ROWS, 0, 1, :],
                id_bf[:TAIL_ROWS, :TAIL_ROWS], start=True, stop=True,
            )
            kt_s = kts.tile([P, 512], BF16, tag="k0")
            copy_out(kt_s[:, :TAIL_ROWS], ktt0[:, :TAIL_ROWS])
            copy_out(kt_s[:, 64:64 + TAIL_ROWS], ktt1[:, :TAIL_ROWS])
            # flush the deferred score MMs (the tail transposes above keep
            # the Tensor engine fed while the last copies complete)
            while pend:
                emit_score(pend.pop(0))
            sct = psum_sc.tile([HG, 512], F32, tag="sc")
            nc.tensor.matmul(
                sct[:H, :TAIL_ROWS], wk32[:, 0, 0, :H], kt_s[:, :TAIL_ROWS],
                start=True, stop=False,
            )
            nc.tensor.matmul(
                sct[:H, :TAIL_ROWS], wk32[:, 1, 0, :H],
                kt_s[:, 64:64 + TAIL_ROWS], start=False, stop=True,
            )
            nc.scalar.activation(
                out=probT[:, :],
                in_=sct[:H, :TAIL_ROWS],
                func=mybir.ActivationFunctionType.Exp,
                bias=qb32[:H, i:i + 1],
                scale=1.0,
                accum_out=sums[:H, NSETS:NSETS + 1],
            )

            # ---- totals, scale, store -------------------------------------
            srow = small.tile([HG, 1], F32, tag="srow")
            nc.vector.reduce_sum(out=srow[:, :], in_=sums[:, :], axis=mybir.AxisListType.X)
            totp = psum_sc.tile([HG, 512], F32, tag="sc")
            nc.tensor.matmul(totp[:, :1], fold32[:, :], srow[:, :])
            rec32 = small.tile([HG, 1], F32, tag="rec32")
            nc.vector.reciprocal(out=rec32[:, :], in_=totp[:, :1])

            # Scale+store as one task per 2048-row segment.  For the first
            # batch the tasks are DEFERRED and dripped one-per-set through
            # the next batch's loop (dumping 11 muls + 11 stores at once
            # collides with the next batch's copies/exps and stalls it by
            # ~10us).  The last batch runs them immediately, muls round-
            # robin DVE/ACT/Pool and stores round-robin sync/gpsimd/scalar.
            if last_batch:
                mul_engs = [
                    lambda seg, np_, r=rec32: nc.vector.tensor_scalar_mul(
                        seg, seg, r[:np_, :]),
                    lambda seg, np_, r=rec32: nc.scalar.mul(seg, seg, r[:np_, :1]),
                ]
                st_engs = [nc.sync, nc.gpsimd, nc.scalar]
            else:
                mul_engs = [
                    lambda seg, np_, r=rec32: nc.vector.tensor_scalar_mul(
                        seg, seg, r[:np_, :]),
                    lambda seg, np_, r=rec32: nc.scalar.mul(seg, seg, r[:np_, :1]),
                ]
                st_engs = [nc.sync]
            pg = PART_S // 4  # 3 groups in the partial set
            full_cols = 512 * FULL_LOADS  # 4608

            def seg_task(mi, i_=i, prob_=prob32, probT_=probT, rec_=rec32,
                         mul_engs_=mul_engs, st_engs_=st_engs):
                if mi < FULL_LOADS:
                    seg = prob_[:, 512 * mi:512 * (mi + 1)]
                    mul_engs_[mi % len(mul_engs_)](seg, HG)
                    st_engs_[mi % len(st_engs_)].dma_start(
                        out=out[i_, :, SROWS * mi:SROWS * (mi + 1)].rearrange(
                            "h (g c) -> g h c", c=512
                        ),
                        in_=seg,
                    )
                elif mi == FULL_LOADS:
                    segp = prob_[:H * pg, full_cols:full_cols + 512]
                    mul_engs_[mi % len(mul_engs_)](segp, H * pg)
                    st_engs_[mi % len(st_engs_)].dma_start(
                        out=out[i_, :, SROWS * FULL_LOADS:TAIL_N0].rearrange(
                            "h (g c) -> g h c", c=512
                        ),
                        in_=segp,
                    )
                else:
                    nc.scalar.mul(probT_[:, :], probT_[:, :], rec_[:H, :1])
                    st_engs_[mi % len(st_engs_)].dma_start(
                        out=out[i_, :, TAIL_N0:], in_=probT_[:, :]
                    )

            for mi in range(FULL_LOADS + 2):
                seg_task(mi)

    nc.compile()
    return nc


_NC_CACHE = {}


def _get_nc():
    if "nc" not in _NC_CACHE:
        _NC_CACHE["nc"] = build_kernel()
    return _NC_CACHE["nc"]


def make_in_maps(query, key, W, b):
    """Host-side precompute of the tiny operands + per-core input maps."""
    query = np.asarray(query, np.float32).reshape(B, D)
    key = np.ascontiguousarray(np.asarray(key, np.float32))
    W = np.asarray(W, np.float32)
    b = np.asarray(b, np.float32)

    Wq, Wk = W[:, :D], W[:, D:]                      # [H, D] each
    qb_all = query @ Wq.T + b[None, :]               # [B, H]
    WkT = Wk.T.astype(NP_BF16)                       # [D, H] bf16
    # wk32[p, dhalf, g, 8g+h] = WkT[dhalf*128 + p, h]
    wk32 = np.zeros((P, 2, G, HG), NP_BF16)
    for g in range(G):
        wk32[:, 0, g, g * H:(g + 1) * H] = WkT[:P]
        wk32[:, 1, g, g * H:(g + 1) * H] = WkT[P:]
    idbf = np.eye(P, dtype=NP_BF16)
    fold32 = np.tile(np.eye(H, dtype=np.float32), (G, G))       # [HG, HG]

    in_maps = []
    for c in range(NCORES):
        s = slice(BPC * c, BPC * (c + 1))
        qb32 = np.tile(np.ascontiguousarray(qb_all[s].T), (G, 1))  # [HG, bpc]
        in_maps.append(
            {
                "k": key[s],
                "idbf": idbf,
                "w32": np.ascontiguousarray(wk32.reshape(P, 2 * G * HG)),
                "qb32": np.ascontiguousarray(qb32),
                "fold32": fold32,
            }
        )
    return in_maps


def unpermute(raw):
    """Undo the p-major within-set n-permutation (device col -> true n).

    Full sets: dev col SROWS*m+512g+128j+p holds true n SROWS*m+64p+4g+j.
    Partial:   dev col 16384+512g+128j+p  holds true n 16384+28p+4g+j.
    Tail (32): already in true order.
    """
    nb, nh = raw.shape[0], raw.shape[1]
    out = np.empty_like(raw)
    blk = raw[:, :, : FULL_LOADS * SROWS].reshape(nb, nh, FULL_LOADS, G, 4, P)
    out[:, :, : FULL_LOADS * SROWS] = blk.transpose(0, 1, 2, 5, 3, 4).reshape(
        nb, nh, FULL_LOADS * SROWS
    )
    pb = raw[:, :, FULL_LOADS * SROWS:TAIL_N0].reshape(nb, nh, PART_S // 4, 4, P)
    out[:, :, FULL_LOADS * SROWS:TAIL_N0] = pb.transpose(0, 1, 4, 2, 3).reshape(
        nb, nh, PART_S * P
    )
    out[:, :, TAIL_N0:] = raw[:, :, TAIL_N0:]
    return out


def kernel(query, key, W, b):
    from concourse.bass_utils import run_bass_kernel_spmd

    nc = _get_nc()
    in_maps = make_in_maps(query, key, W, b)
    res = run_bass_kernel_spmd(nc, in_maps, list(range(NCORES))).results
    raw = np.concatenate([res[c]["out"] for c in range(NCORES)], axis=0)
    return unpermute(raw)


# revision 45
# speedup vs baseline: 1.0877x; 1.0778x over previous
"""Trainium2 Bass kernel: fused concat-linear attention map + softmax.

reference:  scores[b,h,n] = key[b,n,:]@Wk[h,:] + query[b,0,:]@Wq[h,:] + bias[h]
            attn = softmax over n              (B=16, N=20000, D=256, H=8)

v7 structure (per core = 2 batches, each 20000 rows):
  * tiny operands (bf16 identity, zero-padded WkT stationaries, q@Wq+b
    biases, fold32 matrix) are precomputed on the host and DMA'd in.
  * p-MAJOR cast-DMA key loads (SWDGE, f32->bf16 during the HBM read):
    "(p s) d" puts 64 consecutive rows on one partition -> 16KB-chunk
    contiguous HBM reads per partition (vs 1KB in n-interleaved order),
    measurably faster wire (~6-8us/core).  The resulting within-set
    n-permutation (n = 64p + 4g + j) is undone on the HOST during the
    gather/unshard step (pure reshape/transpose, no math).
  * every set-load is split into 2 half-DMAs (first set: 4 quarters)
    so consumers wake on partial data; shortens ramp and tail.
  * 128x128 key-tile transposes as REGULAR bf16 matmuls vs bf16 identity
    (pipelined LDW+MM, FWL; avoids transpose-mode stalls).
  * PSUM->SBUF copies (f32 -> round bf16) split DVE/ACT by ratio.
  * score matmuls use ZERO-PADDED stationaries wk32[dhalf][g] ([128,128],
    cols 8g..8g+8 = WkT half): SIXTEEN 512-col sub-chunks accumulate into
    ONE [128,512] PSUM bank at partition groups g=0..15 (8192 rows/set).
    ACT/DVE op cost depends on FREE size only, so stacking 16 groups on
    the partition axis cuts exp/reduce/scale work 4x vs G=4 -- all
    vector-engine pressure drops below the wire and the batch-boundary
    store burst shrinks to 4 tasks.  (G=4 -> 8 -> 16 measured ~139.5 ->
    132.6 -> 131.3 us fast-core.)
  * score MMs are emitted 2 groups BEHIND their transposes (the in-order
    Tensor engine otherwise stalls on each group's PSUM->SBUF copy).
  * totals: ONE fold32 [32,32] matmul replaces the foldA/foldB round
    trip; reciprocal reads the PSUM result directly; the last set's exp
    accumulates its sum in-instruction (accum_out) to shorten the chain.
  * final scale muls alternate DVE/ACT; store DMAs round-robin
    sync/gpsimd/scalar on the last batch so the tail is not
    issue-serialized on one engine.  (gpsimd/Pool must NEVER run
    tensor math here: its DSP path is ~10x slower and numerically
    broke tensor_scalar in testing.)
"""

import sys

import numpy as np

for _p in ("/opt/trn_rl_repo",):
    if _p not in sys.path:
        sys.path.append(_p)

from contextlib import ExitStack

import ml_dtypes
import concourse.bass as bass
import concourse.bacc as bacc
import concourse.tile as tile
from concourse import mybir

B, N, D, H = 16, 20000, 256, 8
NCORES = 8
BPC = B // NCORES
P = 128
G = 16               # partition groups stacked in score PSUM
HG = H * G           # 32
F32 = mybir.dt.float32
BF16 = mybir.dt.bfloat16
NP_BF16 = ml_dtypes.bfloat16

# per-batch plan: 4 full loads (S=32 subtiles = 4096 rows), one S=28 load,
# then a 32-row tail handled separately.
FULL_LOADS = 2
PART_S = 28          # subtiles in the partial load (3584 rows)
SROWS = 8192         # rows per full set (G=16 partition groups stacked)
TAIL_N0 = FULL_LOADS * SROWS + PART_S * 128  # 19968
TAIL_ROWS = N - TAIL_N0                      # 32
NSETS = FULL_LOADS + 1                       # score psum sets per batch
ACT_COPY_FRAC = 0.44  # fraction of PSUM->SBUF copies done on ACT


def build_kernel(bpc=BPC, reps=1):
    nc = bacc.Bacc("TRN2", target_bir_lowering=False, debug=False)
    k_in = nc.declare_dram_parameter("k", [bpc, N, D], F32, isOutput=False)
    id_in = nc.declare_dram_parameter("idbf", [P, P], BF16, isOutput=False)
    w32_in = nc.declare_dram_parameter("w32", [P, 2 * G * HG], BF16, isOutput=False)
    qb32_in = nc.declare_dram_parameter("qb32", [HG, bpc], F32, isOutput=False)
    fold32_in = nc.declare_dram_parameter("fold32", [HG, HG], F32, isOutput=False)
    out = nc.declare_dram_parameter("out", [bpc, H, N], F32, isOutput=True)

    with ExitStack() as ctx:
        tc = ctx.enter_context(tile.TileContext(nc))
        consts = ctx.enter_context(tc.tile_pool(name="consts", bufs=1))
        loads = ctx.enter_context(tc.tile_pool(name="loads", bufs=5))
        kts = ctx.enter_context(tc.tile_pool(name="kts", bufs=4))
        probp = ctx.enter_context(tc.tile_pool(name="prob", bufs=2))
        small = ctx.enter_context(tc.tile_pool(name="small", bufs=2))
        psum_kt = ctx.enter_context(tc.tile_pool(name="psum_kt", bufs=3, space="PSUM"))
        psum_sc = ctx.enter_context(tc.tile_pool(name="psum_sc", bufs=2, space="PSUM"))

        id_bf = consts.tile([P, P], BF16)
        nc.sync.dma_start(out=id_bf[:, :], in_=id_in[:, :])
        wk32 = consts.tile([P, 2, G, HG], BF16)
        nc.sync.dma_start(
            out=wk32[:, :, :, :],
            in_=w32_in[:, :].rearrange("p (d g x) -> p d g x", d=2, g=G),
        )
        qb32 = consts.tile([HG, bpc], F32)
        nc.sync.dma_start(out=qb32[:, :], in_=qb32_in[:, :])
        fold32 = consts.tile([HG, HG], F32)
        nc.sync.dma_start(out=fold32[:, :], in_=fold32_in[:, :])

        act_period = max(2, round(1.0 / max(ACT_COPY_FRAC, 1e-6)))
        copy_idx = [0]

        def copy_out(dst, src):
            i = copy_idx[0]
            copy_idx[0] += 1
            # first ~1.5 sets: all-DVE so ACT reaches the first exp sooner
            if i >= 12 and i % act_period == 0:
                nc.scalar.copy(out=dst, in_=src)
            else:
                nc.vector.tensor_copy(out=dst, in_=src)

        # --- main loop -----------------------------------------------------
        first_iter = [True]
        pend = []  # deferred score MMs: (scp, m, g, ng, k0, k1, prob32, sums, i)

        def emit_score(e):
            scp_, m_, g_, ng_, k0_, k1_, prob_, sums_, ib_ = e
            nc.tensor.matmul(
                scp_[:, :], wk32[:, 0, g_, :], k0_[:, :],
                start=(g_ == 0), stop=False,
            )
            nc.tensor.matmul(
                scp_[:, :], wk32[:, 1, g_, :], k1_[:, :],
                start=False, stop=(g_ == ng_ - 1),
            )
            if g_ == ng_ - 1:
                hg = ng_ * H
                if m_ == NSETS - 1:
                    # last set: accumulate the sum inside the exp so the
                    # totals chain starts immediately (critical path)
                    nc.scalar.activation(
                        out=prob_[:hg, 512 * m_:512 * (m_ + 1)],
                        in_=scp_[:hg, :],
                        func=mybir.ActivationFunctionType.Exp,
                        bias=qb32[:hg, ib_:ib_ + 1],
                        scale=1.0,
                        accum_out=sums_[:hg, m_:m_ + 1],
                    )
                else:
                    nc.scalar.activation(
                        out=prob_[:hg, 512 * m_:512 * (m_ + 1)],
                        in_=scp_[:hg, :],
                        func=mybir.ActivationFunctionType.Exp,
                        bias=qb32[:hg, ib_:ib_ + 1],
                        scale=1.0,
                    )
                    nc.vector.reduce_sum(
                        out=sums_[:hg, m_:m_ + 1],
                        in_=prob_[:hg, 512 * m_:512 * (m_ + 1)],
                        axis=mybir.AxisListType.X,
                    )

        for i in [ib for _ in range(reps) for ib in range(bpc)]:
            last_batch = i == bpc - 1
            # prob32[8g+h, 512m+c] = attn[h, perm(n)] (pre-scale)
            prob32 = probp.tile([HG, 512 * NSETS], F32, tag="prob")
            probT = probp.tile([H, TAIL_ROWS], F32, tag="probT")
            sums = small.tile([HG, NSETS + 1], F32, tag="sums")
            nc.vector.memset(sums[:, :], 0)

            for m in range(NSETS):
                S = 64 if m < FULL_LOADS else PART_S
                n0 = m * SROWS
                rows = S * P
                kb = loads.tile([P, 64, 2, P], BF16, tag="load")
                # p-major source view: partition p holds rows n0+S*p..+S-1
                src = k_in[i, n0:n0 + rows, :].rearrange("(p s) d -> p s d", p=P)
                if first_iter[0]:
                    first_iter[0] = False
                    bounds = [0, 4, 8, 16, 32, 64]  # fine-grained: short ramp
                elif S != 64:
                    # group-aligned: the last chunk gates only ONE score
                    # group's transposes, shortening the post-wire tail
                    bounds = [0, 8, 16, 20, 24, 28]
                else:
                    bounds = [0, 16, 32, 48, 64]  # quarters elsewhere
                for q in range(len(bounds) - 1):
                    nc.gpsimd.dma_start(
                        out=kb[:, bounds[q]:bounds[q + 1], :, :],
                        in_=src[:, bounds[q]:bounds[q + 1], :],
                    )
                ng = S // 4  # 512-col groups in this set (4 or 3)
                scp = psum_sc.tile([HG, 512], F32, tag="sc")
                for g in range(ng):
                    kt0 = psum_kt.tile([P, 512], F32, tag="kt0")
                    kt1 = psum_kt.tile([P, 512], F32, tag="kt1")
                    for t in range(4):
                        s = g * 4 + t
                        nc.tensor.matmul(
                            kt0[:, t * P:(t + 1) * P],
                            kb[:, s, 0, :],
                            id_bf[:, :],
                            start=True,
                            stop=True,
                        )
                        nc.tensor.matmul(
                            kt1[:, t * P:(t + 1) * P],
                            kb[:, s, 1, :],
                            id_bf[:, :],
                            start=True,
                            stop=True,
                        )
                    k0 = kts.tile([P, 512], BF16, tag="k0")
                    k1 = kts.tile([P, 512], BF16, tag="k1")
                    copy_out(k0[:, :], kt0[:, :])
                    copy_out(k1[:, :], kt1[:, :])
                    # defer the score MMs by 2 groups: the in-order Tensor
                    # engine would otherwise stall on each group's PSUM->SBUF
                    # copy (~0.3us x 80 groups); with the delay the copy
                    # completes during the next 2 groups' transposes.
                    pend.append((scp, m, g, ng, k0, k1, prob32, sums, i))
                    if len(pend) > 2:
                        emit_score(pend.pop(0))


            # ---- 32-row tail (natural n order) ----------------------------
            kbt = loads.tile([P, 16, 2, P], BF16, tag="load")
            nc.gpsimd.dma_start(
                out=kbt[:TAIL_ROWS, :1, :, :],
                in_=k_in[i, TAIL_N0:N, :].rearrange("(s p) d -> p s d", p=TAIL_ROWS),
            )
            ktt0 = psum_kt.tile([P, 512], F32, tag="kt0")
            ktt1 = psum_kt.tile([P, 512], F32, tag="kt1")
            nc.tensor.matmul(
                ktt0[:, :TAIL_ROWS], kbt[:TAIL_ROWS, 0, 0, :],
                id_bf[:TAIL_ROWS, :TAIL_ROWS], start=True, stop=True,
            )
            nc.tensor.matmul(
                ktt1[:, :TAIL_ROWS], kbt[:TAIL_ROWS, 0, 1, :],
                id_bf[:TAIL_ROWS, :TAIL_ROWS], start=True, stop=True,
            )
            kt_s = kts.tile([P, 512], BF16, tag="k0")
            copy_out(kt_s[:, :TAIL_ROWS], ktt0[:, :TAIL_ROWS])
            copy_out(kt_s[:, 64:64 + TAIL_ROWS], ktt1[:, :TAIL_ROWS])
            # flush the deferred score MMs (the tail transposes above keep
            # the Tensor engine fed while the last copies complete)
            while pend:
                emit_score(pend.pop(0))
            sct = psum_sc.tile([HG, 512], F32, tag="sc")
            nc.tensor.matmul(
                sct[:H, :TAIL_ROWS], wk32[:, 0, 0, :H], kt_s[:, :TAIL_ROWS],
                start=True, stop=False,
            )
            nc.tensor.matmul(
                sct[:H, :TAIL_ROWS], wk32[:, 1, 0, :H],
                kt_s[:, 64:64 + TAIL_ROWS], start=False, stop=True,
            )
            nc.scalar.activation(
                out=probT[:, :],
                in_=sct[:H, :TAIL_ROWS],
                func=mybir.ActivationFunctionType.Exp,
                bias=qb32[:H, i:i + 1],
                scale=1.0,
                accum_out=sums[:H, NSETS:NSETS + 1],
            )

            # ---- totals, scale, store -------------------------------------
            srow = small.tile([HG, 1], F32, tag="srow")
            nc.vector.reduce_sum(out=srow[:, :], in_=sums[:, :], axis=mybir.AxisListType.X)
            totp = psum_sc.tile([HG, 512], F32, tag="sc")
            nc.tensor.matmul(totp[:, :1], fold32[:, :], srow[:, :])
            rec32 = small.tile([HG, 1], F32, tag="rec32")
            nc.vector.reciprocal(out=rec32[:, :], in_=totp[:, :1])

            # Scale+store as one task per 2048-row segment.  For the first
            # batch the tasks are DEFERRED and dripped one-per-set through
            # the next batch's loop (dumping 11 muls + 11 stores at once
            # collides with the next batch's copies/exps and stalls it by
            # ~10us).  The last batch runs them immediately, muls round-
            # robin DVE/ACT/Pool and stores round-robin sync/gpsimd/scalar.
            if last_batch:
                mul_engs = [
                    lambda seg, np_, r=rec32: nc.vector.tensor_scalar_mul(
                        seg, seg, r[:np_, :]),
                    lambda seg, np_, r=rec32: nc.scalar.mul(seg, seg, r[:np_, :1]),
                ]
                st_engs = [nc.sync, nc.gpsimd, nc.scalar]
            else:
                mul_engs = [
                    lambda seg, np_, r=rec32: nc.vector.tensor_scalar_mul(
                        seg, seg, r[:np_, :]),
                    lambda seg, np_, r=rec32: nc.scalar.mul(seg, seg, r[:np_, :1]),
                ]
                st_engs = [nc.sync]
            pg = PART_S // 4  # 3 groups in the partial set
            full_cols = 512 * FULL_LOADS  # 4608

            def seg_task(mi, i_=i, prob_=prob32, probT_=probT, rec_=rec32,
                         mul_engs_=mul_engs, st_engs_=st_engs):
                if mi < FULL_LOADS:
                    seg = prob_[:, 512 * mi:512 * (mi + 1)]
                    mul_engs_[mi % len(mul_engs_)](seg, HG)
                    st_engs_[mi % len(st_engs_)].dma_start(
                        out=out[i_, :, SROWS * mi:SROWS * (mi + 1)].rearrange(
                            "h (g c) -> g h c", c=512
                        ),
                        in_=seg,
                    )
                elif mi == FULL_LOADS:
                    segp = prob_[:H * pg, full_cols:full_cols + 512]
                    mul_engs_[mi % len(mul_engs_)](segp, H * pg)
                    st_engs_[mi % len(st_engs_)].dma_start(
                        out=out[i_, :, SROWS * FULL_LOADS:TAIL_N0].rearrange(
                            "h (g c) -> g h c", c=512
                        ),
                        in_=segp,
                    )
                else:
                    nc.scalar.mul(probT_[:, :], probT_[:, :], rec_[:H, :1])
                    st_engs_[mi % len(st_engs_)].dma_start(
                        out=out[i_, :, TAIL_N0:], in_=probT_[:, :]
                    )

            for mi in range(FULL_LOADS + 2):
                seg_task(mi)

    nc.compile()
    return nc


_NC_CACHE = {}


def _get_nc():
    if "nc" not in _NC_CACHE:
        _NC_CACHE["nc"] = build_kernel()
    return _NC_CACHE["nc"]


def make_in_maps(query, key, W, b):
    """Host-side precompute of the tiny operands + per-core input maps."""
    query = np.asarray(query, np.float32).reshape(B, D)
    key = np.ascontiguousarray(np.asarray(key, np.float32))
    W = np.asarray(W, np.float32)
    b = np.asarray(b, np.float32)

    Wq, Wk = W[:, :D], W[:, D:]                      # [H, D] each
    qb_all = query @ Wq.T + b[None, :]               # [B, H]
    WkT = Wk.T.astype(NP_BF16)                       # [D, H] bf16
    # wk32[p, dhalf, g, 8g+h] = WkT[dhalf*128 + p, h]
    wk32 = np.zeros((P, 2, G, HG), NP_BF16)
    for g in range(G):
        wk32[:, 0, g, g * H:(g + 1) * H] = WkT[:P]
        wk32[:, 1, g, g * H:(g + 1) * H] = WkT[P:]
    idbf = np.eye(P, dtype=NP_BF16)
    fold32 = np.tile(np.eye(H, dtype=np.float32), (G, G))       # [HG, HG]

    in_maps = []
    for c in range(NCORES):
        s = slice(BPC * c, BPC * (c + 1))
        qb32 = np.tile(np.ascontiguousarray(qb_all[s].T), (G, 1))  # [HG, bpc]
        in_maps.append(
            {
                "k": key[s],
                "idbf": idbf,
                "w32": np.ascontiguousarray(wk32.reshape(P, 2 * G * HG)),
                "qb32": np.ascontiguousarray(qb32),
                "fold32": fold32,
            }
        )
    return in_maps


def unpermute(raw):
    """Undo the p-major within-set n-permutation (device col -> true n).

    Full sets: dev col SROWS*m+512g+128j+p holds true n SROWS*m+64p+4g+j.
    Partial:   dev col 16384+512g+128j+p  holds true n 16384+28p+4g+j.
    Tail (32): already in true order.
    """
    nb, nh = raw.shape[0], raw.shape[1]
    out = np.empty_like(raw)
    blk = raw[:, :, : FULL_LOADS * SROWS].reshape(nb, nh, FULL_LOADS, G, 4, P)
    out[:, :, : FULL_LOADS * SROWS] = blk.transpose(0, 1, 2, 5, 3, 4).reshape(
        nb, nh, FULL_LOADS * SROWS
    )
    pb = raw[:, :, FULL_LOADS * SROWS:TAIL_N0].reshape(nb, nh, PART_S // 4, 4, P)
    out[:, :, FULL_LOADS * SROWS:TAIL_N0] = pb.transpose(0, 1, 4, 2, 3).reshape(
        nb, nh, PART_S * P
    )
    out[:, :, TAIL_N0:] = raw[:, :, TAIL_N0:]
    return out


def kernel(query, key, W, b):
    from concourse.bass_utils import run_bass_kernel_spmd

    nc = _get_nc()
    in_maps = make_in_maps(query, key, W, b)
    res = run_bass_kernel_spmd(nc, in_maps, list(range(NCORES))).results
    raw = np.concatenate([res[c]["out"] for c in range(NCORES)], axis=0)
    return unpermute(raw)
